# revision 44
# baseline (speedup 1.0000x reference)
"""Trainium2 Bass kernel for AdaBiDiff GNN message passing.

Data parallel over batch B=8, one batch element per core.  Per core:
  xt (12,1536) -> softmax over t -> p, logp (t-major)
  kl[i,j] = rowterm[i] - sum_t p[i,t] logp[j,t];  A = (kl < 0.5)
  u_fwd = (A @ xt.T) / rowsum(A);  u_bwd = (A.T @ xt.T) / colsum(A)
  x_flat[n, t*64+h] = relu(xt[t,n] W1[h] + (0.9 u_fwd + 2.1 u_bwd)[n,t] W2[h])
  two MLP blocks (BN folded into weights on host) -> out (12,1536) per core.

Implementation notes:
  - all weights are baked into the NEFF as inline Const tensors (loaded to
    HBM once at model-load time); the only per-call transfers are x in and
    out back.  The jitted SPMD executable and device-resident zero output
    buffers are cached across kernel() calls; weight content changes are
    detected by fingerprint and trigger a rebuild.
  - one packed (128, CW) f32 weight blob -> a single in-kernel DMA; each
    weight is an SBUF column-slice view of the blob.
  - augmented-G: phat=[p;0..;rowterm-0.5], lhat=[logp;0..;-1] at partitions
    0..11 and 32, so Ghat = 0.5-kl and A = (Ghat > 0).  phat/lhat duplicated
    at partitions 64..96 so the two Ghat orientations run row-packed
    (tile_position (0,0) vs (64,0)) concurrently on the PE.
  - A-orientation compare on DVE (is_gt -> 0/1); AT-orientation on ScalarE
    (Sign -> -1/0/1), with the sign-affine correction folded into the
    u_fwd scaling: yA=(yA'+Sx)/2, rs=(rs'+N)/2 -> uf=(yA'+Sx)/(rs'+N).
  - ones column in the transposed-x stationary produces row/col sums free.
  - both product accumulators share PSUM banks (partitions 0-32 and 64-96
    of the same tiles) -> 3 banks, letting Ghat tiles double-buffer.
  - x_flat build via one K=76 matmul per (k,c): moving operand xs stacks
    [xt;0;uf;0;ub] at partitions 0-75 (uf/ub written in place by stage C),
    stationary stacks [e1t;0;e2a;0;e2b] inside the weight blob.  Zero gap
    rows make the 32-alignment rule happy and cost no PE time (matmul time
    scales with N, not K).
  - matmul dtype float32r (1 col/cycle); A/AT tiles and xtT in bf16.
"""

import numpy as np

import concourse.bass as bass
import concourse.bacc as bacc
import concourse.tile as tile
import concourse.mybir as mybir

F32 = mybir.dt.float32
F32R = mybir.dt.float32r
BF16 = mybir.dt.bfloat16
AF = mybir.ActivationFunctionType
ALU = mybir.AluOpType

B, T, N, H, TH, HID2, TOUT = 8, 12, 1536, 64, 768, 128, 12
NT = N // 128
NC = N // 512
AUG = 32

# ---- packed weight blob column layout ----
O_EW1 = 0              # 6 x 128 cols, rows 0-127
O_EPROJ = 768          # 6 x 64 cols, rows 0-127
O_EW2 = 1152           # 128 cols, rows 0-127
O_EW3 = 1280           # 64 cols, rows 0-127
O_DW1 = 1344           # 128 cols, rows 0-63
O_DW2 = 1472           # 128 cols, rows 0-127
O_DW3 = 1600           # 12 cols, rows 0-127
O_DPROJ = 1612         # 12 cols, rows 0-63
O_ES = 1624            # 768 cols, rows 0-75 ([e1t;0;e2a;0;e2b] stack)
O_I12 = 2392           # 12 cols, rows 0-11 (identity)
O_EB1 = 2404           # bias columns (f32 bits)
O_EB2 = 2405
O_EBE = 2406
O_DB1 = 2407
O_DB2 = 2408
O_DBD = 2409
CW = 2410

_cache = {}


def _build_nc(wblob):
    nc = bacc.Bacc("TRN2", target_bir_lowering=False, debug=False)
    d = {}
    d["x"] = nc.declare_dram_parameter("x", [T, N], F32R, isOutput=False)
    d["out"] = nc.declare_dram_parameter("out", [T, N], F32, isOutput=True)
    d["wb"] = nc.inline_tensor(wblob, name="wb")
    # xs rows 12..63 fill: zeros with a ones row where row 33 lands
    zc1 = np.zeros((52, N), np.float32)
    zc1[33 - 12, :] = 1.0
    d["zc1"] = nc.inline_tensor(zc1, name="zc1")
    # xs rows 76..97 fill: zeros with a ones row where row 97 lands
    zc2 = np.zeros((22, N), np.float32)
    zc2[97 - 76, :] = 1.0
    d["zc2"] = nc.inline_tensor(zc2, name="zc2")

    with tile.TileContext(nc) as tc:
        _kernel_body(tc, d)
    nc.compile()
    return nc


def _kernel_body(tc, d):
    nc = tc.nc
    CS = [slice(c * 512, (c + 1) * 512) for c in range(NC)]

    with tc.tile_pool(name="w", bufs=1) as w, tc.tile_pool(name="sb", bufs=1) as sb:

        def stile(name, shape, dt=F32R):
            return sb.tile(list(shape), dt, name=name, tag=name)

        # ---- per-call input + weight blob (x first: it gates the whole chain) ----
        # xs doubles as the Ghat j-side operand AND the stage-D moving stack:
        #   rows 0-11 xt | 12-31 zero | 32 -L -> uf | 33 one -> uf | 34-43 zero
        #   -> uf | 44-63 zero | 64-75 xt-dup -> ub | 76-95 zero | 96 -L-dup |
        #   97 one-dup.  (uf/ub overwrite the B-side rows only after stage B.)
        xs = stile("xs", (98, N))
        nc.sync.dma_start(out=xs[0:T, :], in_=d["x"].ap())
        wb = w.tile([128, CW], F32R, name="wb", tag="wb")
        nc.sync.dma_start(out=wb[:].bitcast(F32), in_=d["wb"].ap())
        xt = xs[0:T, :]
        i12 = wb[0:T, O_I12:O_I12 + T]
        eb1 = wb[:, O_EB1:O_EB1 + 1].bitcast(F32)
        eb2 = wb[:, O_EB2:O_EB2 + 1].bitcast(F32)
        ebe = wb[0:H, O_EBE:O_EBE + 1].bitcast(F32)
        db1 = wb[:, O_DB1:O_DB1 + 1].bitcast(F32)
        db2 = wb[:, O_DB2:O_DB2 + 1].bitcast(F32)
        dbd = wb[0:TOUT, O_DBD:O_DBD + 1].bitcast(F32)

        ones12 = w.tile([T, 1], F32R, name="ones12", tag="ones12")
        nc.vector.memset(ones12[:].bitcast(F32), 1.0)
        ones1 = w.tile([1, T], F32R, name="ones1", tag="ones1")
        nc.vector.memset(ones1[:].bitcast(F32), 1.0)
        # stage-C staging bias column: rows 0-11 = Sx (filled later), row 32 = N
        bSx = w.tile([33, 1], F32, name="bSx", tag="bSx")
        nc.vector.memset(bSx[:], 0.0)
        nc.vector.memset(bSx[32:33, :], float(N))
        # prewarm the exp activation table under the input DMAs
        warm = w.tile([1, 1], F32, name="warm", tag="warm")
        nc.vector.memset(warm[:], 1.0)
        nc.scalar.activation(warm[:], warm[:], AF.Exp)

        # =========== Stage A ===========
        # Ghat is computed SCALED by s[i] = Sum_t ex[t,i] > 0 (compare vs 0 is
        # unchanged):  s*Ghat[i,j] = Sum_t ex[t,i]*x[t,j]
        #                          + s[i]*(-L[j]) + ((0.5+L[i])*s[i] - W[i])*1
        # with L = ln(s), W = Sum_t ex*x.  So the K=34 operand stacks are raw
        # rows: phat = [ex; 0..; s@32; combo@33], xs = [xt; 0..; -L@32; 1@33],
        # both duplicated at partitions 64..97 for PE row-packing.
        phat = stile("phat", (98, N))
        xtT = stile("xtT", (128, NT, AUG + 1), BF16)

        nc.gpsimd.memset(phat[0:33, :].bitcast(F32), 0.0)
        nc.gpsimd.memset(phat[64:97, :].bitcast(F32), 0.0)
        nc.gpsimd.memset(xtT[:], 0.0)
        # xs zero/one fills (rows 12-63, 76-97) and the xt dup at 64-75
        nc.gpsimd.dma_start(out=xs[T:64, :].bitcast(F32), in_=d["zc1"].ap())
        nc.gpsimd.dma_start(out=xs[76:98, :].bitcast(F32), in_=d["zc2"].ap())
        nc.sync.dma_start(out=xs[64:76, :], in_=d["x"].ap())

        with tc.tile_pool(name="pa1", bufs=1, space="PSUM") as pa1, \
             tc.tile_pool(name="pat", bufs=1, space="PSUM") as pat:
            nc.scalar.activation(phat[0:T, :], xt, AF.Exp)
            # hidden Ln-table load while the psA matmuls run
            nc.scalar.activation(warm[:], warm[:], AF.Ln)
            wx = stile("wx", (T, N))
            nc.vector.tensor_tensor(wx[:], phat[0:T, :], xt, ALU.mult)
            # duplicate ex rows for the row-packed orientation
            nc.sync.dma_start(out=phat[64:76, :], in_=phat[0:T, :])

            psA = pa1.tile([1, NC, 512], F32, name="psA", tag="psA")
            psW = pa1.tile([1, NC, 512], F32, name="psW", tag="psW")
            for c in range(NC):
                nc.tensor.matmul(psA[:, c, :], ones12[:], phat[0:T, CS[c]],
                                 start=True, stop=True)
            for c in range(NC):
                nc.tensor.matmul(psW[:, c, :], ones12[:], wx[:, CS[c]],
                                 start=True, stop=True)

            # augP stages [s @ row 0; combo @ row 32]; one strided 2-row DMA per
            # row-packing copy then lands them at phat rows 32-33 / 96-97.
            L = stile("L", (1, N), F32)
            cmb = stile("cmb", (1, N), F32)
            augP = stile("augP", (33, N), F32)
            nc.scalar.activation(L[:], psA[:], AF.Ln)
            nc.scalar.activation(augP[0:1, :], psA[:], AF.Identity)
            # combo = (0.5 + L)*s - W
            nc.vector.tensor_scalar(cmb[:], L[:], 0.5, None, ALU.add)
            nc.vector.tensor_tensor(cmb[:], cmb[:], augP[0:1, :], ALU.mult)
            nc.vector.tensor_tensor(augP[32:33, :], cmb[:], psW[:], ALU.subtract)
            nc.sync.dma_start(out=phat[32:34, :].bitcast(F32), in_=augP[0:33:32, :])
            nc.scalar.dma_start(out=phat[96:98, :].bitcast(F32), in_=augP[0:33:32, :])
            # xs aug row 32: -L (both row-packing copies; rows 33/97 are const 1)
            nc.vector.tensor_scalar(xs[AUG:AUG + 1, :], L[:], -1.0, None, ALU.mult)
            nc.vector.tensor_scalar(xs[96:97, :], L[:], -1.0, None, ALU.mult)

            # transposed x with ones column (bf16): xtT[p, j, t] = xt[t, 128j+p]
            psT = pat.tile([128, NT, T], F32, name="psT", tag="psT")
            for j in range(NT):
                nc.tensor.matmul(psT[:, j, :], xt[:, j * 128:(j + 1) * 128], i12,
                                 start=True, stop=True)
            nc.vector.tensor_copy(xtT[:, :, 0:T], psT[:])
            nc.vector.memset(xtT[:, :, AUG:AUG + 1], 1.0)

        # =========== Stage B: Ghat, adjacency, products ===========
        with tc.tile_pool(name="pp", bufs=1, space="PSUM") as pp, \
             tc.tile_pool(name="ab", bufs=3) as ab:

            # shared-bank product accumulator: rows 0..32 = [yA';rs'] (sign),
            # rows 64..96 = [yAT;cs] (0/1); one 3-bank tile, bank per chunk
            prod = pp.tile([128, NC, 512], F32, name="prod", tag="prod")

            with tc.tile_pool(name="pgg", bufs=3, space="PSUM") as pgg, \
                 tc.tile_pool(name="pgt", bufs=2, space="PSUM") as pgt:
                tiles = []
                for i in range(NT):
                    Ai = ab.tile([128, N], BF16, name="Ai", tag="Ai")
                    ATi = ab.tile([128, N], BF16, name="ATi", tag="ATi")
                    isl = slice(i * 128, (i + 1) * 128)
                    for c in range(NC):
                        psG = pgg.tile([128, 512], F32, name="psG", tag="psG")
                        nc.tensor.matmul(psG[:], phat[0:34, isl], xs[0:34, CS[c]],
                                         start=True, stop=True, tile_position=(0, 0))
                        nc.vector.tensor_scalar(Ai[:, CS[c]], psG[:], 0.0, None, ALU.is_gt)
                        psGT = pgt.tile([128, 512], F32, name="psGT", tag="psGT")
                        nc.tensor.matmul(psGT[:], xs[64:98, isl], phat[64:98, CS[c]],
                                         start=True, stop=True, tile_position=(64, 0))
                        nc.scalar.sign(ATi[:, CS[c]], psGT[:])
                    # software-pipelined: emit products for i-1 after Ghat(i)
                    tiles.append((Ai, ATi))
                    for j in ([i - 1] if i > 0 else []) + ([i] if i == NT - 1 else []):
                        Aj, ATj = tiles[j]
                        for c in range(NC):
                            nc.tensor.matmul(prod[0:AUG + 1, c, :], xtT[:, j, :], ATj[:, CS[c]],
                                             start=(j == 0), stop=(j == NT - 1),
                                             skip_group_check=True, tile_position=(0, 0))
                            nc.tensor.matmul(prod[64:97, c, :], xtT[:, j, :], Aj[:, CS[c]],
                                             start=(j == 0), stop=(j == NT - 1),
                                             skip_group_check=True, tile_position=(0, 64))

            # ===== Stage C: scale products into xs rows 32-43 (uf), 64-75 (ub) =====
            # uf = (yA' + Sx) / (rs' + N)   [sign-corrected];  ub = yAT / cs
            # Each prod bank is read by exactly ONE engine (Act) in two wide
            # staging passes: rows carry [data+Sx; rs'+N] / [data; cs] via a
            # per-partition bias column.
            nc.vector.tensor_reduce(bSx[0:T, :], xt, mybir.AxisListType.X, ALU.add)
            vf = stile("vf", (33, N), F32)
            vb = stile("vb", (33, N), F32)
            nc.scalar.activation(vf[:], prod[0:33, :, :], AF.Identity, bias=bSx[:])
            nc.scalar.activation(vb[:], prod[64:97, :, :], AF.Identity)

        # prod pool closed; reciprocals, PE row-broadcast, multiply into xs
        with tc.tile_pool(name="pc", bufs=1, space="PSUM") as pc:
            rr = stile("rr", (1, N), F32R)
            cc = stile("cc", (1, N), F32R)
            with nc.allow_low_precision(reason="4-byte recips"):
                nc.vector.reciprocal(rr[:], vf[32:33, :])
                nc.vector.reciprocal(cc[:], vb[32:33, :])
            rrB = pc.tile([T, NC, 512], F32, name="rrB", tag="rrB")
            ccB = pc.tile([T, NC, 512], F32, name="ccB", tag="ccB")
            for c in range(NC):
                nc.tensor.matmul(rrB[:, c, :], ones1[:], rr[:, CS[c]], start=True, stop=True)
                nc.tensor.matmul(ccB[:, c, :], ones1[:], cc[:, CS[c]], start=True, stop=True)
            nc.vector.tensor_tensor(xs[32:32 + T, :], vf[0:T, :], rrB[:], ALU.mult)
            nc.vector.tensor_tensor(xs[64:64 + T, :], vb[0:T, :], ccB[:], ALU.mult)

        # =========== Stages D/E/F ===========
        # zT: independent k-slices -> wide 3-bank tiles + 1536-wide post-ops.
        # h1..od: serial data chain -> per-chunk tiles keep it pipelined.
        zT = stile("zT", (128, 6, N))
        with tc.tile_pool(name="pfz", bufs=2, space="PSUM") as pfz:
            for k in range(6):
                ps = pfz.tile([128, NC, 512], F32, name="psF", tag="psz")
                for c in range(NC):
                    nc.tensor.matmul(ps[:, c, :], wb[0:76, O_ES + k * 128:O_ES + (k + 1) * 128],
                                     xs[0:76, CS[c]], start=True, stop=True)
                if k % 2 == 0:
                    nc.scalar.activation(zT[:, k, :], ps[:], AF.Relu)
                else:
                    nc.vector.tensor_scalar(zT[:, k, :], ps[:], 0.0, None, ALU.max)

        with tc.tile_pool(name="pf", bufs=6, space="PSUM") as pf:
            h1 = stile("h1", (HID2, N))
            for c in range(NC):
                ps = pf.tile([HID2, 512], F32, name="psH1", tag="ps")
                for k in range(6):
                    nc.tensor.matmul(ps[:], wb[:, O_EW1 + k * 128:O_EW1 + (k + 1) * 128],
                                     zT[:, k, CS[c]], start=(k == 0), stop=(k == 5))
                if c % 2 == 0:
                    nc.scalar.activation(h1[:, CS[c]], ps[:], AF.Relu, bias=eb1)
                else:
                    nc.vector.tensor_scalar(h1[:, CS[c]], ps[:], eb1, 0.0, ALU.add, ALU.max)

            h2 = stile("h2", (HID2, N))
            for c in range(NC):
                ps = pf.tile([HID2, 512], F32, name="psH2", tag="ps")
                nc.tensor.matmul(ps[:], wb[:, O_EW2:O_EW2 + HID2], h1[:, CS[c]],
                                 start=True, stop=True)
                if c % 2 == 1:
                    nc.scalar.activation(h2[:, CS[c]], ps[:], AF.Relu, bias=eb2)
                else:
                    nc.vector.tensor_scalar(h2[:, CS[c]], ps[:], eb2, 0.0, ALU.add, ALU.max)

            xe = stile("xe", (H, N))
            for c in range(NC):
                ps = pf.tile([H, 512], F32, name="psXe", tag="ps")
                nc.tensor.matmul(ps[:], wb[:, O_EW3:O_EW3 + H], h2[:, CS[c]],
                                 start=True, stop=False)
                for k in range(6):
                    nc.tensor.matmul(ps[:], wb[:, O_EPROJ + k * H:O_EPROJ + (k + 1) * H],
                                     zT[:, k, CS[c]], start=False, stop=(k == 5))
                if c % 2 == 0:
                    nc.scalar.activation(xe[:, CS[c]], ps[:], AF.Identity, bias=ebe)
                else:
                    nc.vector.tensor_scalar(xe[:, CS[c]], ps[:], ebe, None, ALU.add)

            g1 = stile("g1", (HID2, N))
            for c in range(NC):
                ps = pf.tile([HID2, 512], F32, name="psG1", tag="ps")
                nc.tensor.matmul(ps[:], wb[0:H, O_DW1:O_DW1 + HID2], xe[:, CS[c]],
                                 start=True, stop=True)
                if c % 2 == 1:
                    nc.scalar.activation(g1[:, CS[c]], ps[:], AF.Relu, bias=db1)
                else:
                    nc.vector.tensor_scalar(g1[:, CS[c]], ps[:], db1, 0.0, ALU.add, ALU.max)

            g2 = stile("g2", (HID2, N))
            for c in range(NC):
                ps = pf.tile([HID2, 512], F32, name="psG2", tag="ps")
                nc.tensor.matmul(ps[:], wb[:, O_DW2:O_DW2 + HID2], g1[:, CS[c]],
                                 start=True, stop=True)
                if c % 2 == 0:
                    nc.scalar.activation(g2[:, CS[c]], ps[:], AF.Relu, bias=db2)
                else:
                    nc.vector.tensor_scalar(g2[:, CS[c]], ps[:], db2, 0.0, ALU.add, ALU.max)

            od = stile("od", (TOUT, N), F32)
            for c in range(NC):
                ps = pf.tile([TOUT, 512], F32, name="psOd", tag="ps")
                nc.tensor.matmul(ps[:], wb[:, O_DW3:O_DW3 + TOUT], g2[:, CS[c]],
                                 start=True, stop=False)
                nc.tensor.matmul(ps[:], wb[0:H, O_DPROJ:O_DPROJ + TOUT], xe[:, CS[c]],
                                 start=False, stop=True)
                if c % 2 == 1:
                    nc.scalar.activation(od[:, CS[c]], ps[:], AF.Identity, bias=dbd)
                else:
                    nc.vector.tensor_scalar(od[:, CS[c]], ps[:], dbd, None, ALU.add)
                eng = (nc.sync, nc.scalar, nc.gpsimd)[c]
                eng.dma_start(out=d["out"].ap()[:, CS[c]], in_=od[:, CS[c]])


def _build_wblob(inputs):
    f32 = np.float32
    W1 = np.asarray(inputs["W1"], f32)[0]
    W2 = np.asarray(inputs["W2"], f32)[0]
    g = np.asarray(inputs["enc_bn_g"], f32); be = np.asarray(inputs["enc_bn_b"], f32)
    m = np.asarray(inputs["enc_bn_m"], f32); v = np.asarray(inputs["enc_bn_v"], f32)
    esc = g / np.sqrt(v + 1e-5)
    ew3 = np.asarray(inputs["enc_w3"], f32) * esc[None, :]
    eproj = np.asarray(inputs["enc_proj"], f32) * esc[None, :]
    ebe = np.asarray(inputs["enc_b3"], f32) * esc + (be - m * esc)
    g = np.asarray(inputs["dec_bn_g"], f32); bd = np.asarray(inputs["dec_bn_b"], f32)
    m = np.asarray(inputs["dec_bn_m"], f32); v = np.asarray(inputs["dec_bn_v"], f32)
    dsc = g / np.sqrt(v + 1e-5)
    dw3 = np.asarray(inputs["dec_w3"], f32) * dsc[None, :]
    dproj = np.asarray(inputs["dec_proj"], f32) * dsc[None, :]
    dbd = np.asarray(inputs["dec_b3"], f32) * dsc + (bd - m * dsc)

    wb = np.zeros((128, CW), f32)
    ew1 = np.asarray(inputs["enc_w1"], f32)
    for a in range(6):
        wb[:, O_EW1 + a * 128:O_EW1 + (a + 1) * 128] = ew1[a * 128:(a + 1) * 128, :]
        wb[:, O_EPROJ + a * H:O_EPROJ + (a + 1) * H] = eproj[a * 128:(a + 1) * 128, :]
    wb[:, O_EW2:O_EW2 + HID2] = np.asarray(inputs["enc_w2"], f32)
    wb[:, O_EW3:O_EW3 + H] = ew3
    wb[0:H, O_DW1:O_DW1 + HID2] = np.asarray(inputs["dec_w1"], f32)
    wb[:, O_DW2:O_DW2 + HID2] = np.asarray(inputs["dec_w2"], f32)
    wb[:, O_DW3:O_DW3 + TOUT] = dw3
    wb[0:H, O_DPROJ:O_DPROJ + TOUT] = dproj
    # [e1t;0;e2a;0;e2b] stack: block-diagonal W rows per t
    for t in range(T):
        wb[t, O_ES + t * H:O_ES + (t + 1) * H] = W1
        wb[32 + t, O_ES + t * H:O_ES + (t + 1) * H] = 0.9 * W2    # K_HOPS * ALPHA
        wb[64 + t, O_ES + t * H:O_ES + (t + 1) * H] = 2.1 * W2    # K_HOPS * (1-ALPHA)
    wb[0:T, O_I12:O_I12 + T] = np.eye(T, dtype=f32)
    wb[:, O_EB1] = np.asarray(inputs["enc_b1"], f32)
    wb[:, O_EB2] = np.asarray(inputs["enc_b2"], f32)
    wb[0:H, O_EBE] = ebe
    wb[:, O_DB1] = np.asarray(inputs["dec_b1"], f32)
    wb[:, O_DB2] = np.asarray(inputs["dec_b2"], f32)
    wb[0:TOUT, O_DBD] = dbd
    return wb


def _weights_fp(inputs):
    """Content fingerprint of every non-x input (cheap; full-content hash)."""
    import hashlib
    h = hashlib.blake2b(digest_size=16)
    for k in sorted(inputs):
        if k == "x":
            continue
        a = np.ascontiguousarray(np.asarray(inputs[k]))
        h.update(k.encode())
        h.update(str(a.shape).encode())
        h.update(a.tobytes())
    return h.digest()


def _make_runner(nc):
    import jax
    from jax.sharding import Mesh, PartitionSpec, NamedSharding
    from jax.experimental.shard_map import shard_map
    from concourse.bass2jax import (_bass_exec_p, install_neuronx_cc_hook,
                                    partition_id_tensor)

    install_neuronx_cc_hook()
    partition_name = nc.partition_id_tensor.name if nc.partition_id_tensor else None

    in_names, out_names, out_avals, zero_shapes = [], [], [], []
    for alloc in nc.m.functions[0].allocations:
        if not isinstance(alloc, mybir.MemoryLocationSet):
            continue
        name = alloc.memorylocations[0].name
        if alloc.kind == "ExternalInput":
            if name != partition_name:
                in_names.append(name)
        elif alloc.kind == "ExternalOutput":
            out_names.append(name)
            shape = tuple(alloc.tensor_shape)
            dtype = mybir.dt.np(alloc.dtype)
            out_avals.append(jax.core.ShapedArray(shape, dtype))
            zero_shapes.append((shape, dtype))
    n_params = len(in_names)
    all_in_names = tuple(in_names + out_names + ([partition_name] if partition_name else []))

    def _body(*args):
        operands = list(args)
        if partition_name is not None:
            operands.append(partition_id_tensor())
        outs = _bass_exec_p.bind(
            *operands,
            out_avals=tuple(out_avals),
            in_names=all_in_names,
            out_names=tuple(out_names),
            lowering_input_output_aliases=(),
            sim_require_finite=True,
            sim_require_nnan=True,
            nc=nc,
        )
        return tuple(outs)

    devices = jax.devices()[:B]
    mesh = Mesh(np.asarray(devices), ("core",))
    nin = n_params + len(out_names)
    sharded = jax.jit(
        shard_map(_body, mesh=mesh, in_specs=(PartitionSpec("core"),) * nin,
                  out_specs=(PartitionSpec("core"),) * len(out_names), check_rep=False),
        keep_unused=True,
    )
    sh = NamedSharding(mesh, PartitionSpec("core"))
    zeros = [jax.device_put(np.zeros((B * s[0], *s[1:]), dt), sh)
             for (s, dt) in zero_shapes]
    return sharded, zeros


def _build_ctx(inputs):
    wb = _build_wblob(inputs)
    nc = _build_nc(wb)
    sharded, zeros = _make_runner(nc)
    return {"fp": _weights_fp(inputs), "nc": nc, "sharded": sharded, "zeros": zeros,
            "ids": tuple(id(inputs[k]) for k in sorted(inputs) if k != "x")}


def kernel(**inputs) -> np.ndarray:
    ctx = _cache.get("ctx")
    if ctx is not None:
        ids = tuple(id(inputs[k]) for k in sorted(inputs) if k != "x")
        if ids != ctx["ids"]:
            if _weights_fp(inputs) == ctx["fp"]:
                ctx["ids"] = ids
            else:
                ctx = None
    if ctx is None:
        ctx = _build_ctx(inputs)
        _cache["ctx"] = ctx

    x = np.asarray(inputs["x"], np.float32).reshape(B * T, N)
    out = ctx["sharded"](x, *ctx["zeros"])[0]
    return np.asarray(out).reshape(B, TOUT, N, 1).astype(np.float32, copy=False)


# revision 69
# speedup vs baseline: 2.0670x; 2.0670x over previous
"""Trainium2 Bass kernel for AdaBiDiff GNN message passing.

Data parallel over batch B=8, one batch element per core.  Per core:
  xt (12,1536) -> softmax over t -> p, logp (t-major)
  kl[i,j] = rowterm[i] - sum_t p[i,t] logp[j,t];  A = (kl < 0.5)
  u_fwd = (A @ xt.T) / rowsum(A);  u_bwd = (A.T @ xt.T) / colsum(A)
  x_flat[n, t*64+h] = relu(xt[t,n] W1[h] + (0.9 u_fwd + 2.1 u_bwd)[n,t] W2[h])
  two MLP blocks (BN folded into weights on host) -> out (12,1536) per core.

Implementation notes:
  - all weights are baked into the NEFF as inline Const tensors (loaded to
    HBM once at model-load time); the only per-call transfers are x in and
    out back.  The jitted SPMD executable and device-resident zero output
    buffers are cached across kernel() calls; weight content changes are
    detected by fingerprint (id fast path) and trigger a rebuild.
  - one packed (128, CW) f32 weight blob -> a single in-kernel DMA; each
    weight is an SBUF column-slice view of the blob.
  - the KL adjacency compare is computed SCALED by s[i] = Sum_t exp(x[t,i])
    (> 0, so A = (Ghat > 0) is unchanged):
      s*Ghat[i,j] = Sum_t ex[t,i]x[t,j] + s[i]*(-L[j]) + cmb[i]*1,
    with L = ln(s), cmb = (0.5+L)*s - W, W = Sum_t ex*x.  The K=34 operand
    stacks are therefore raw rows: phat = [ex;0..;s@32;cmb@33] and
    xs = [xt;0..;-L@32;1@33], duplicated at partitions 64..97 so the two
    orientations run row-packed (tile_position (0,0) vs (64,0)) on the PE.
    s/cmb land at rows 32-33/96-97 via partition-strided 2-row DMAs; the
    softmax itself (p, logp) is never materialized.
  - A-orientation compare on DVE (is_gt -> 0/1); AT-orientation on ScalarE
    (Sign -> -1/0/1), with the sign-affine correction folded into the
    u_fwd scaling: yA=(yA'+Sx)/2, rs=(rs'+N)/2 -> uf=(yA'+Sx)/(rs'+N).
  - ones column in the transposed-x stationary produces row/col sums free.
  - both product accumulators share PSUM banks (partitions 0-32 and 64-96
    of one 3-bank tile), letting the Ghat tiles double-buffer.
  - stages B-F run as a per-512-column-chunk pipeline: chunk c+1's
    PE-heavy adjacency/products overlap chunk c's DVE/Act-heavy
    normalization and MLP tail.  uf/ub land in a separate xd stack
    ([xt;0;uf;0;ub], rows 0-75) so the writes never collide with stage-B
    reads of xs; x_flat is one K=76 matmul per (k,c) against the
    [e1t;0;e2a;0;e2b] stack in the blob (zero gap rows satisfy the
    32-alignment rule and cost no PE time - matmul time scales with N,
    not K).
  - matmul dtype float32r (1 col/cycle); A/AT tiles and xtT in bf16.
"""

import numpy as np

import concourse.bass as bass
import concourse.bacc as bacc
import concourse.tile as tile
import concourse.mybir as mybir

F32 = mybir.dt.float32
F32R = mybir.dt.float32r
BF16 = mybir.dt.bfloat16
AF = mybir.ActivationFunctionType
ALU = mybir.AluOpType

B, T, N, H, TH, HID2, TOUT = 8, 12, 1536, 64, 768, 128, 12
NT = N // 128
NC = N // 512
AUG = 32

# ---- packed weight blob column layout ----
O_EW1 = 0              # 6 x 128 cols, rows 0-127
O_EPROJ = 768          # 6 x 64 cols, rows 0-127
O_EW2 = 1152           # 128 cols, rows 0-127
O_EW3 = 1280           # 64 cols, rows 0-127
O_DW1 = 1344           # 128 cols, rows 0-63
O_DW2 = 1472           # 128 cols, rows 0-127
O_DW3 = 1600           # 12 cols, rows 0-127
O_DPROJ = 1612         # 12 cols, rows 0-63
O_ES = 1624            # 768 cols, rows 0-75 ([e1t;0;e2a;0;e2b] stack)
O_I12 = 2392           # 12 cols, rows 0-11 (identity)
O_EB1 = 2404           # bias columns (f32 bits)
O_EB2 = 2405
O_EBE = 2406
O_DB1 = 2407
O_DB2 = 2408
O_DBD = 2409
CW = 2410

_cache = {}


def _build_nc(wblob):
    nc = bacc.Bacc("TRN2", target_bir_lowering=False, debug=False)
    d = {}
    d["x"] = nc.declare_dram_parameter("x", [T, N], F32R, isOutput=False)
    d["out"] = nc.declare_dram_parameter("out", [T, N], F32, isOutput=True)
    d["wb"] = nc.inline_tensor(wblob, name="wb")
    # xs rows 12..63 fill: zeros with a ones row where row 33 lands
    zc1 = np.zeros((52, N), np.float32)
    zc1[33 - 12, :] = 1.0
    d["zc1"] = nc.inline_tensor(zc1, name="zc1")
    # xs rows 76..97 fill: zeros with a ones row where row 97 lands
    zc2 = np.zeros((22, N), np.float32)
    zc2[97 - 76, :] = 1.0
    d["zc2"] = nc.inline_tensor(zc2, name="zc2")

    with tile.TileContext(nc) as tc:
        _kernel_body(tc, d)
    nc.compile()
    return nc


def _kernel_body(tc, d):
    nc = tc.nc
    CS = [slice(c * 512, (c + 1) * 512) for c in range(NC)]

    with tc.tile_pool(name="w", bufs=1) as w, tc.tile_pool(name="sb", bufs=1) as sb:

        def stile(name, shape, dt=F32R):
            return sb.tile(list(shape), dt, name=name, tag=name)

        # ---- per-call input + weight blob (x first: it gates the whole chain) ----
        # xs doubles as the Ghat j-side operand AND the stage-D moving stack:
        #   rows 0-11 xt | 12-31 zero | 32 -L -> uf | 33 one -> uf | 34-43 zero
        #   -> uf | 44-63 zero | 64-75 xt-dup -> ub | 76-95 zero | 96 -L-dup |
        #   97 one-dup.  (uf/ub overwrite the B-side rows only after stage B.)
        xs = stile("xs", (98, N))
        nc.sync.dma_start(out=xs[0:T, :], in_=d["x"].ap())
        wb = w.tile([128, CW], F32R, name="wb", tag="wb")
        nc.sync.dma_start(out=wb[:].bitcast(F32), in_=d["wb"].ap())
        xt = xs[0:T, :]
        i12 = wb[0:T, O_I12:O_I12 + T]
        eb1 = wb[:, O_EB1:O_EB1 + 1].bitcast(F32)
        eb2 = wb[:, O_EB2:O_EB2 + 1].bitcast(F32)
        ebe = wb[0:H, O_EBE:O_EBE + 1].bitcast(F32)
        db1 = wb[:, O_DB1:O_DB1 + 1].bitcast(F32)
        db2 = wb[:, O_DB2:O_DB2 + 1].bitcast(F32)
        dbd = wb[0:TOUT, O_DBD:O_DBD + 1].bitcast(F32)

        ones12 = w.tile([T, 1], F32R, name="ones12", tag="ones12")
        nc.vector.memset(ones12[:].bitcast(F32), 1.0)
        ones1 = w.tile([1, T], F32R, name="ones1", tag="ones1")
        nc.vector.memset(ones1[:].bitcast(F32), 1.0)
        # stage-C staging bias column: rows 0-11 = Sx (filled later), row 32 = N
        bSx = w.tile([33, 1], F32, name="bSx", tag="bSx")
        nc.vector.memset(bSx[:], 0.0)
        nc.vector.memset(bSx[32:33, :], float(N))
        ph5 = w.tile([1, 1], F32, name="ph5", tag="ph5")
        nc.vector.memset(ph5[:], 0.5)
        # prewarm the exp activation table under the input DMAs
        warm = w.tile([1, 1], F32, name="warm", tag="warm")
        nc.vector.memset(warm[:], 1.0)
        nc.scalar.activation(warm[:], warm[:], AF.Exp)

        # =========== Stage A ===========
        # Ghat is computed SCALED by s[i] = Sum_t ex[t,i] > 0 (compare vs 0 is
        # unchanged):  s*Ghat[i,j] = Sum_t ex[t,i]*x[t,j]
        #                          + s[i]*(-L[j]) + ((0.5+L[i])*s[i] - W[i])*1
        # with L = ln(s), W = Sum_t ex*x.  So the K=34 operand stacks are raw
        # rows: phat = [ex; 0..; s@32; combo@33], xs = [xt; 0..; -L@32; 1@33],
        # both duplicated at partitions 64..97 for PE row-packing.
        phat = stile("phat", (98, N))
        xtT = stile("xtT", (128, NT, AUG + 1), BF16)

        nc.gpsimd.memset(phat[0:33, :].bitcast(F32), 0.0)
        nc.gpsimd.memset(phat[64:97, :].bitcast(F32), 0.0)
        nc.gpsimd.memset(xtT[:], 0.0)
        # xs zero/one fills (rows 12-63, 76-97) and the xt dup at 64-75
        nc.gpsimd.dma_start(out=xs[T:64, :].bitcast(F32), in_=d["zc1"].ap())
        nc.gpsimd.dma_start(out=xs[76:98, :].bitcast(F32), in_=d["zc2"].ap())
        nc.sync.dma_start(out=xs[64:76, :], in_=d["x"].ap())
        # xd: stage-D moving stack [xt; 0; uf; 0; ub] — separate from xs so
        # per-chunk uf/ub writes don't collide with stage-B reads of xs
        xd = stile("xd", (76, N))
        nc.gpsimd.memset(xd[:].bitcast(F32), 0.0)
        nc.scalar.dma_start(out=xd[0:T, :], in_=d["x"].ap())

        with tc.tile_pool(name="pa1", bufs=1, space="PSUM") as pa1, \
             tc.tile_pool(name="pat", bufs=1, space="PSUM") as pat:
            nc.scalar.activation(phat[0:T, :], xt, AF.Exp)
            # hidden Ln-table load while the psA matmuls run
            nc.scalar.activation(warm[:], warm[:], AF.Ln)
            wx = stile("wx", (T, N))
            nc.vector.tensor_tensor(wx[:], phat[0:T, :], xt, ALU.mult)
            # duplicate ex rows for the row-packed orientation
            nc.sync.dma_start(out=phat[64:76, :], in_=phat[0:T, :])

            psA = pa1.tile([1, NC, 512], F32, name="psA", tag="psA")
            psW = pa1.tile([1, NC, 512], F32, name="psW", tag="psW")
            for c in range(NC):
                nc.tensor.matmul(psA[:, c, :], ones12[:], phat[0:T, CS[c]],
                                 start=True, stop=True)
            for c in range(NC):
                nc.tensor.matmul(psW[:, c, :], ones12[:], wx[:, CS[c]],
                                 start=True, stop=True)

            # augP stages [s @ row 0; combo @ row 32]; one strided 2-row DMA per
            # row-packing copy then lands them at phat rows 32-33 / 96-97.
            L = stile("L", (1, N), F32)
            cmb = stile("cmb", (1, N), F32)
            augP = stile("augP", (33, N), F32)
            nc.scalar.activation(L[:], psA[:], AF.Ln)
            nc.scalar.activation(augP[0:1, :], psA[:], AF.Identity)
            # combo = (0.5 + L)*s - W  (chain first: it gates the aug-row DMAs;
            # L2 = L+0.5 also feeds the -L rows so they schedule after it)
            nc.vector.tensor_scalar(cmb[:], L[:], 0.5, None, ALU.add)
            nc.vector.tensor_tensor(augP[32:33, :], cmb[:], augP[0:1, :], ALU.mult)
            nc.vector.tensor_tensor(augP[32:33, :], augP[32:33, :], psW[:], ALU.subtract)
            nc.sync.dma_start(out=phat[32:34, :].bitcast(F32), in_=augP[0:33:32, :])
            nc.scalar.dma_start(out=phat[96:98, :].bitcast(F32), in_=augP[0:33:32, :])
            # xs aug row 32: -L (both row-packing copies; on Act to keep the
            # DVE free for the combo chain that gates the aug-row DMAs)
            nc.scalar.activation(xs[AUG:AUG + 1, :], L[:], AF.Identity, scale=-1.0)
            nc.scalar.activation(xs[96:97, :], L[:], AF.Identity, scale=-1.0)

            # transposed x with ones column (bf16): xtT[p, j, t] = xt[t, 128j+p]
            psT = pat.tile([128, NT, T], F32, name="psT", tag="psT")
            for j in range(NT):
                nc.tensor.matmul(psT[:, j, :], xt[:, j * 128:(j + 1) * 128], i12,
                                 start=True, stop=True)
            nc.vector.tensor_copy(xtT[:, :, 0:T], psT[:])
            nc.vector.memset(xtT[:, :, AUG:AUG + 1], 1.0)

        # =========== Stages B-F: per-chunk pipeline ===========
        # Each 512-column chunk runs adjacency+products (B), normalization
        # (C), x_flat (D) and the MLPs (E/F) independently, so chunk c+1's
        # PE-heavy stage B overlaps chunk c's DVE/Act-heavy tail.  uf/ub go
        # into the separate xd stack (xs stays read-only after stage A).
        nc.vector.tensor_reduce(bSx[0:T, :], xt, mybir.AxisListType.X, ALU.add)
        zT = stile("zT", (128, 6, N))
        h1 = stile("h1", (HID2, N))
        h2 = stile("h2", (HID2, N))
        xe = stile("xe", (H, N))
        g1 = stile("g1", (HID2, N))
        g2 = stile("g2", (HID2, N))
        od = stile("od", (TOUT, N), F32)
        vf = stile("vf", (33, N), F32)
        vb = stile("vb", (33, N), F32)
        rr = stile("rr", (1, N), F32R)
        cc = stile("cc", (1, N), F32R)

        with tc.tile_pool(name="pp", bufs=2, space="PSUM") as pp, \
             tc.tile_pool(name="ab", bufs=5) as ab, \
             tc.tile_pool(name="pgg", bufs=2, space="PSUM") as pgg, \
             tc.tile_pool(name="pgt", bufs=2, space="PSUM") as pgt, \
             tc.tile_pool(name="pf", bufs=2, space="PSUM") as pf:
            for c in range(NC):
                # ---- B(c): Ghat both orientations, compares, products ----
                prodc = pp.tile([128, 512], F32, name="prodc", tag="prod")
                for i in range(NT):
                    isl = slice(i * 128, (i + 1) * 128)
                    Ai = ab.tile([128, 512], BF16, name="Ai", tag="Ai")
                    ATi = ab.tile([128, 512], BF16, name="ATi", tag="ATi")
                    psG = pgg.tile([128, 512], F32, name="psG", tag="psG")
                    nc.tensor.matmul(psG[:], phat[0:34, isl], xs[0:34, CS[c]],
                                     start=True, stop=True, tile_position=(0, 0))
                    nc.vector.tensor_scalar(Ai[:], psG[:], 0.0, None, ALU.is_gt)
                    psGT = pgt.tile([128, 512], F32, name="psGT", tag="psGT")
                    nc.tensor.matmul(psGT[:], xs[64:98, isl], phat[64:98, CS[c]],
                                     start=True, stop=True, tile_position=(64, 0))
                    nc.scalar.sign(ATi[:], psGT[:])
                    nc.tensor.matmul(prodc[0:33, :], xtT[:, i, :], ATi[:],
                                     start=(i == 0), stop=(i == NT - 1),
                                     skip_group_check=True, tile_position=(0, 0))
                    nc.tensor.matmul(prodc[64:97, :], xtT[:, i, :], Ai[:],
                                     start=(i == 0), stop=(i == NT - 1),
                                     skip_group_check=True, tile_position=(0, 64))

                # ---- C(c): uf = (yA'+Sx)/(rs'+N), ub = yAT/cs -> xd rows ----
                nc.scalar.activation(vf[:, CS[c]], prodc[0:33, :], AF.Identity, bias=bSx[:])
                with nc.allow_low_precision(reason="4-byte recips"):
                    nc.vector.reciprocal(rr[:, CS[c]], vf[32:33, CS[c]])
                rrB = pf.tile([T, 512], F32, name="rrB", tag="ps")
                nc.tensor.matmul(rrB[:], ones1[:], rr[:, CS[c]], start=True, stop=True)
                nc.scalar.activation(vb[:, CS[c]], prodc[64:97, :], AF.Identity)
                with nc.allow_low_precision(reason="4-byte recips"):
                    nc.vector.reciprocal(cc[:, CS[c]], vb[32:33, CS[c]])
                nc.vector.tensor_tensor(xd[32:44, CS[c]], vf[0:T, CS[c]], rrB[:], ALU.mult)
                ccB = pf.tile([T, 512], F32, name="ccB", tag="ps")
                nc.tensor.matmul(ccB[:], ones1[:], cc[:, CS[c]], start=True, stop=True)
                nc.vector.tensor_tensor(xd[64:76, CS[c]], vb[0:T, CS[c]], ccB[:], ALU.mult)

                # ---- D(c): x_flat slices ----
                for k in range(6):
                    ps = pf.tile([128, 512], F32, name="psF", tag="ps")
                    nc.tensor.matmul(ps[:], wb[0:76, O_ES + k * 128:O_ES + (k + 1) * 128],
                                     xd[0:76, CS[c]], start=True, stop=True)
                    if k % 2 == 0:
                        nc.scalar.activation(zT[:, k, CS[c]], ps[:], AF.Relu)
                    else:
                        nc.vector.tensor_scalar(zT[:, k, CS[c]], ps[:], 0.0, None, ALU.max)

                # ---- E/F(c): encoder/decoder MLPs ----
                ps = pf.tile([HID2, 512], F32, name="psH1", tag="ps")
                for k in range(6):
                    nc.tensor.matmul(ps[:], wb[:, O_EW1 + k * 128:O_EW1 + (k + 1) * 128],
                                     zT[:, k, CS[c]], start=(k == 0), stop=(k == 5))
                if c % 2 == 0:
                    nc.scalar.activation(h1[:, CS[c]], ps[:], AF.Relu, bias=eb1)
                else:
                    nc.vector.tensor_scalar(h1[:, CS[c]], ps[:], eb1, 0.0, ALU.add, ALU.max)

                ps = pf.tile([HID2, 512], F32, name="psH2", tag="ps")
                nc.tensor.matmul(ps[:], wb[:, O_EW2:O_EW2 + HID2], h1[:, CS[c]],
                                 start=True, stop=True)
                if c % 2 == 1:
                    nc.scalar.activation(h2[:, CS[c]], ps[:], AF.Relu, bias=eb2)
                else:
                    nc.vector.tensor_scalar(h2[:, CS[c]], ps[:], eb2, 0.0, ALU.add, ALU.max)

                ps = pf.tile([H, 512], F32, name="psXe", tag="ps")
                nc.tensor.matmul(ps[:], wb[:, O_EW3:O_EW3 + H], h2[:, CS[c]],
                                 start=True, stop=False)
                for k in range(6):
                    nc.tensor.matmul(ps[:], wb[:, O_EPROJ + k * H:O_EPROJ + (k + 1) * H],
                                     zT[:, k, CS[c]], start=False, stop=(k == 5))
                if c % 2 == 0:
                    nc.scalar.activation(xe[:, CS[c]], ps[:], AF.Identity, bias=ebe)
                else:
                    nc.vector.tensor_scalar(xe[:, CS[c]], ps[:], ebe, None, ALU.add)

                ps = pf.tile([HID2, 512], F32, name="psG1", tag="ps")
                nc.tensor.matmul(ps[:], wb[0:H, O_DW1:O_DW1 + HID2], xe[:, CS[c]],
                                 start=True, stop=True)
                if c % 2 == 1:
                    nc.scalar.activation(g1[:, CS[c]], ps[:], AF.Relu, bias=db1)
                else:
                    nc.vector.tensor_scalar(g1[:, CS[c]], ps[:], db1, 0.0, ALU.add, ALU.max)

                ps = pf.tile([HID2, 512], F32, name="psG2", tag="ps")
                nc.tensor.matmul(ps[:], wb[:, O_DW2:O_DW2 + HID2], g1[:, CS[c]],
                                 start=True, stop=True)
                if c % 2 == 0:
                    nc.scalar.activation(g2[:, CS[c]], ps[:], AF.Relu, bias=db2)
                else:
                    nc.vector.tensor_scalar(g2[:, CS[c]], ps[:], db2, 0.0, ALU.add, ALU.max)

                ps = pf.tile([TOUT, 512], F32, name="psOd", tag="ps")
                nc.tensor.matmul(ps[:], wb[:, O_DW3:O_DW3 + TOUT], g2[:, CS[c]],
                                 start=True, stop=False)
                nc.tensor.matmul(ps[:], wb[0:H, O_DPROJ:O_DPROJ + TOUT], xe[:, CS[c]],
                                 start=False, stop=True)
                if c % 2 == 1:
                    nc.scalar.activation(od[:, CS[c]], ps[:], AF.Identity, bias=dbd)
                else:
                    nc.vector.tensor_scalar(od[:, CS[c]], ps[:], dbd, None, ALU.add)
                eng = (nc.gpsimd, nc.scalar, nc.sync)[c]
                eng.dma_start(out=d["out"].ap()[:, CS[c]], in_=od[:, CS[c]])


# revision 71
# speedup vs baseline: 40.3888x; 19.5396x over previous
"""Trainium2 Bass kernel for AdaBiDiff GNN message passing.

Data parallel over batch B=8, one batch element per core.  Per core:
  xt (12,1536) -> softmax over t -> p, logp (t-major)
  kl[i,j] = rowterm[i] - sum_t p[i,t] logp[j,t];  A = (kl < 0.5)
  u_fwd = (A @ xt.T) / rowsum(A);  u_bwd = (A.T @ xt.T) / colsum(A)
  x_flat[n, t*64+h] = relu(xt[t,n] W1[h] + (0.9 u_fwd + 2.1 u_bwd)[n,t] W2[h])
  two MLP blocks (BN folded into weights on host) -> out (12,1536) per core.

Implementation notes:
  - all weights are baked into the NEFF as inline Const tensors (loaded to
    HBM once at model-load time); the only per-call transfers are x in and
    out back.  The jitted SPMD executable and device-resident zero output
    buffers are cached across kernel() calls; weight content changes are
    detected by fingerprint (id fast path) and trigger a rebuild.
  - one packed (128, CW) f32 weight blob -> a single in-kernel DMA; each
    weight is an SBUF column-slice view of the blob.
  - the KL adjacency compare is computed SCALED by s[i] = Sum_t exp(x[t,i])
    (> 0, so A = (Ghat > 0) is unchanged):
      s*Ghat[i,j] = Sum_t ex[t,i]x[t,j] + s[i]*(-L[j]) + cmb[i]*1,
    with L = ln(s), cmb = (0.5+L)*s - W, W = Sum_t ex*x.  The K=34 operand
    stacks are therefore raw rows: phat = [ex;0..;s@32;cmb@33] and
    xs = [xt;0..;-L@32;1@33], duplicated at partitions 64..97 so the two
    orientations run row-packed (tile_position (0,0) vs (64,0)) on the PE.
    s/cmb land at rows 32-33/96-97 via partition-strided 2-row DMAs; the
    softmax itself (p, logp) is never materialized.
  - A-orientation compare on DVE (is_gt -> 0/1); AT-orientation on ScalarE
    (Sign -> -1/0/1), with the sign-affine correction folded into the
    u_fwd scaling: yA=(yA'+Sx)/2, rs=(rs'+N)/2 -> uf=(yA'+Sx)/(rs'+N).
  - ones column in the transposed-x stationary produces row/col sums free.
  - both product accumulators share PSUM banks (partitions 0-32 and 64-96
    of one 3-bank tile), letting the Ghat tiles double-buffer.
  - stages B-F run as a per-512-column-chunk pipeline: chunk c+1's
    PE-heavy adjacency/products overlap chunk c's DVE/Act-heavy
    normalization and MLP tail.  uf/ub land in a separate xd stack
    ([xt;0;uf;0;ub], rows 0-75) so the writes never collide with stage-B
    reads of xs; x_flat is one K=76 matmul per (k,c) against the
    [e1t;0;e2a;0;e2b] stack in the blob (zero gap rows satisfy the
    32-alignment rule and cost no PE time - matmul time scales with N,
    not K).
  - matmul dtype float32r (1 col/cycle); A/AT tiles and xtT in bf16.
"""

import numpy as np

import concourse.bass as bass
import concourse.bacc as bacc
import concourse.tile as tile
import concourse.mybir as mybir

F32 = mybir.dt.float32
F32R = mybir.dt.float32r
BF16 = mybir.dt.bfloat16
AF = mybir.ActivationFunctionType
ALU = mybir.AluOpType

B, T, N, H, TH, HID2, TOUT = 8, 12, 1536, 64, 768, 128, 12
NT = N // 128
NC = N // 512
AUG = 32

# ---- packed weight blob column layout ----
O_EW1 = 0              # 6 x 128 cols, rows 0-127
O_EPROJ = 768          # 6 x 64 cols, rows 0-127
O_EW2 = 1152           # 128 cols, rows 0-127
O_EW3 = 1280           # 64 cols, rows 0-127
O_DW1 = 1344           # 128 cols, rows 0-63
O_DW2 = 1472           # 128 cols, rows 0-127
O_DW3 = 1600           # 12 cols, rows 0-127
O_DPROJ = 1612         # 12 cols, rows 0-63
O_ES = 1624            # 768 cols, rows 0-75 ([e1t;0;e2a;0;e2b] stack)
O_I12 = 2392           # 12 cols, rows 0-11 (identity)
O_EB1 = 2404           # bias columns (f32 bits)
O_EB2 = 2405
O_EBE = 2406
O_DB1 = 2407
O_DB2 = 2408
O_DBD = 2409
CW = 2410

_cache = {}


def _build_nc(wblob):
    nc = bacc.Bacc("TRN2", target_bir_lowering=False, debug=False)
    d = {}
    d["x"] = nc.declare_dram_parameter("x", [T, N], F32R, isOutput=False)
    d["out"] = nc.declare_dram_parameter("out", [T, N], F32, isOutput=True)
    d["wb"] = nc.inline_tensor(wblob, name="wb")
    # xs rows 12..63 fill: zeros with a ones row where row 33 lands
    zc1 = np.zeros((52, N), np.float32)
    zc1[33 - 12, :] = 1.0
    d["zc1"] = nc.inline_tensor(zc1, name="zc1")
    # xs rows 76..97 fill: zeros with a ones row where row 97 lands
    zc2 = np.zeros((22, N), np.float32)
    zc2[97 - 76, :] = 1.0
    d["zc2"] = nc.inline_tensor(zc2, name="zc2")

    with tile.TileContext(nc) as tc:
        _kernel_body(tc, d)
    nc.compile()
    return nc


def _kernel_body(tc, d):
    nc = tc.nc
    CS = [slice(c * 512, (c + 1) * 512) for c in range(NC)]

    with tc.tile_pool(name="w", bufs=1) as w, tc.tile_pool(name="sb", bufs=1) as sb:

        def stile(name, shape, dt=F32R):
            return sb.tile(list(shape), dt, name=name, tag=name)

        # ---- per-call input + weight blob (x first: it gates the whole chain) ----
        # xs doubles as the Ghat j-side operand AND the stage-D moving stack:
        #   rows 0-11 xt | 12-31 zero | 32 -L -> uf | 33 one -> uf | 34-43 zero
        #   -> uf | 44-63 zero | 64-75 xt-dup -> ub | 76-95 zero | 96 -L-dup |
        #   97 one-dup.  (uf/ub overwrite the B-side rows only after stage B.)
        xs = stile("xs", (98, N))
        nc.sync.dma_start(out=xs[0:T, :], in_=d["x"].ap())
        wb = w.tile([128, CW], F32R, name="wb", tag="wb")
        nc.sync.dma_start(out=wb[:].bitcast(F32), in_=d["wb"].ap())
        xt = xs[0:T, :]
        i12 = wb[0:T, O_I12:O_I12 + T]
        eb1 = wb[:, O_EB1:O_EB1 + 1].bitcast(F32)
        eb2 = wb[:, O_EB2:O_EB2 + 1].bitcast(F32)
        ebe = wb[0:H, O_EBE:O_EBE + 1].bitcast(F32)
        db1 = wb[:, O_DB1:O_DB1 + 1].bitcast(F32)
        db2 = wb[:, O_DB2:O_DB2 + 1].bitcast(F32)
        dbd = wb[0:TOUT, O_DBD:O_DBD + 1].bitcast(F32)

        ones12 = w.tile([T, 1], F32R, name="ones12", tag="ones12")
        nc.vector.memset(ones12[:].bitcast(F32), 1.0)
        ones1 = w.tile([1, T], F32R, name="ones1", tag="ones1")
        nc.vector.memset(ones1[:].bitcast(F32), 1.0)
        # stage-C staging bias column: rows 0-11 = Sx (filled later), row 32 = N
        bSx = w.tile([33, 1], F32, name="bSx", tag="bSx")
        nc.vector.memset(bSx[:], 0.0)
        nc.vector.memset(bSx[32:33, :], float(N))
        ph5 = w.tile([1, 1], F32, name="ph5", tag="ph5")
        nc.vector.memset(ph5[:], 0.5)
        # prewarm the exp activation table under the input DMAs
        warm = w.tile([1, 1], F32, name="warm", tag="warm")
        nc.vector.memset(warm[:], 1.0)
        nc.scalar.activation(warm[:], warm[:], AF.Exp)

        # =========== Stage A ===========
        # Ghat is computed SCALED by s[i] = Sum_t ex[t,i] > 0 (compare vs 0 is
        # unchanged):  s*Ghat[i,j] = Sum_t ex[t,i]*x[t,j]
        #                          + s[i]*(-L[j]) + ((0.5+L[i])*s[i] - W[i])*1
        # with L = ln(s), W = Sum_t ex*x.  So the K=34 operand stacks are raw
        # rows: phat = [ex; 0..; s@32; combo@33], xs = [xt; 0..; -L@32; 1@33],
        # both duplicated at partitions 64..97 for PE row-packing.
        phat = stile("phat", (98, N))
        xtT = stile("xtT", (128, NT, AUG + 1), BF16)

        nc.gpsimd.memset(phat[0:33, :].bitcast(F32), 0.0)
        nc.gpsimd.memset(phat[64:97, :].bitcast(F32), 0.0)
        nc.gpsimd.memset(xtT[:], 0.0)
        # xs zero/one fills (rows 12-63, 76-97) and the xt dup at 64-75
        nc.gpsimd.dma_start(out=xs[T:64, :].bitcast(F32), in_=d["zc1"].ap())
        nc.gpsimd.dma_start(out=xs[76:98, :].bitcast(F32), in_=d["zc2"].ap())
        nc.sync.dma_start(out=xs[64:76, :], in_=d["x"].ap())
        # xd: stage-D moving stack [xt; 0; uf; 0; ub] — separate from xs so
        # per-chunk uf/ub writes don't collide with stage-B reads of xs
        xd = stile("xd", (76, N))
        nc.gpsimd.memset(xd[:].bitcast(F32), 0.0)
        nc.scalar.dma_start(out=xd[0:T, :], in_=d["x"].ap())

        with tc.tile_pool(name="pa1", bufs=1, space="PSUM") as pa1, \
             tc.tile_pool(name="pat", bufs=1, space="PSUM") as pat:
            nc.scalar.activation(phat[0:T, :], xt, AF.Exp)
            # hidden Ln-table load while the psA matmuls run
            nc.scalar.activation(warm[:], warm[:], AF.Ln)
            wx = stile("wx", (T, N))
            nc.vector.tensor_tensor(wx[:], phat[0:T, :], xt, ALU.mult)
            # duplicate ex rows for the row-packed orientation
            nc.sync.dma_start(out=phat[64:76, :], in_=phat[0:T, :])

            psA = pa1.tile([1, NC, 512], F32, name="psA", tag="psA")
            psW = pa1.tile([1, NC, 512], F32, name="psW", tag="psW")
            for c in range(NC):
                nc.tensor.matmul(psA[:, c, :], ones12[:], phat[0:T, CS[c]],
                                 start=True, stop=True)
            for c in range(NC):
                nc.tensor.matmul(psW[:, c, :], ones12[:], wx[:, CS[c]],
                                 start=True, stop=True)

            # augP stages [s @ row 0; combo @ row 32]; one strided 2-row DMA per
            # row-packing copy then lands them at phat rows 32-33 / 96-97.
            L = stile("L", (1, N), F32)
            cmb = stile("cmb", (1, N), F32)
            augP = stile("augP", (33, N), F32)
            nc.scalar.activation(L[:], psA[:], AF.Ln)
            nc.scalar.activation(augP[0:1, :], psA[:], AF.Identity)
            # combo = (0.5 + L)*s - W  (chain first: it gates the aug-row DMAs;
            # L2 = L+0.5 also feeds the -L rows so they schedule after it)
            nc.vector.tensor_scalar(cmb[:], L[:], 0.5, None, ALU.add)
            nc.vector.tensor_tensor(augP[32:33, :], cmb[:], augP[0:1, :], ALU.mult)
            nc.vector.tensor_tensor(augP[32:33, :], augP[32:33, :], psW[:], ALU.subtract)
            nc.sync.dma_start(out=phat[32:34, :].bitcast(F32), in_=augP[0:33:32, :])
            nc.scalar.dma_start(out=phat[96:98, :].bitcast(F32), in_=augP[0:33:32, :])
            # xs aug row 32: -L (both row-packing copies; on Act to keep the
            # DVE free for the combo chain that gates the aug-row DMAs)
            nc.scalar.activation(xs[AUG:AUG + 1, :], L[:], AF.Identity, scale=-1.0)
            nc.scalar.activation(xs[96:97, :], L[:], AF.Identity, scale=-1.0)

            # transposed x with ones column (bf16): xtT[p, j, t] = xt[t, 128j+p]
            psT = pat.tile([128, NT, T], F32, name="psT", tag="psT")
            for j in range(NT):
                nc.tensor.matmul(psT[:, j, :], xt[:, j * 128:(j + 1) * 128], i12,
                                 start=True, stop=True)
            nc.vector.tensor_copy(xtT[:, :, 0:T], psT[:])
            nc.vector.memset(xtT[:, :, AUG:AUG + 1], 1.0)

        # =========== Stages B-F: per-chunk pipeline ===========
        # Each 512-column chunk runs adjacency+products (B), normalization
        # (C), x_flat (D) and the MLPs (E/F) independently, so chunk c+1's
        # PE-heavy stage B overlaps chunk c's DVE/Act-heavy tail.  uf/ub go
        # into the separate xd stack (xs stays read-only after stage A).
        nc.vector.tensor_reduce(bSx[0:T, :], xt, mybir.AxisListType.X, ALU.add)
        zT = stile("zT", (128, 6, N))
        h1 = stile("h1", (HID2, N))
        h2 = stile("h2", (HID2, N))
        xe = stile("xe", (H, N))
        g1 = stile("g1", (HID2, N))
        g2 = stile("g2", (HID2, N))
        od = stile("od", (TOUT, N), F32)
        vf = stile("vf", (33, N), F32)
        vb = stile("vb", (33, N), F32)
        rr = stile("rr", (1, N), F32R)
        cc = stile("cc", (1, N), F32R)

        with tc.tile_pool(name="pp", bufs=2, space="PSUM") as pp, \
             tc.tile_pool(name="ab", bufs=5) as ab, \
             tc.tile_pool(name="pgg", bufs=2, space="PSUM") as pgg, \
             tc.tile_pool(name="pgt", bufs=2, space="PSUM") as pgt, \
             tc.tile_pool(name="pf", bufs=2, space="PSUM") as pf:
            for c in range(NC):
                # ---- B(c): Ghat both orientations, compares, products ----
                prodc = pp.tile([128, 512], F32, name="prodc", tag="prod")
                for i in range(NT):
                    isl = slice(i * 128, (i + 1) * 128)
                    Ai = ab.tile([128, 512], BF16, name="Ai", tag="Ai")
                    ATi = ab.tile([128, 512], BF16, name="ATi", tag="ATi")
                    psG = pgg.tile([128, 512], F32, name="psG", tag="psG")
                    nc.tensor.matmul(psG[:], phat[0:34, isl], xs[0:34, CS[c]],
                                     start=True, stop=True, tile_position=(0, 0))
                    nc.vector.tensor_scalar(Ai[:], psG[:], 0.0, None, ALU.is_gt)
                    psGT = pgt.tile([128, 512], F32, name="psGT", tag="psGT")
                    nc.tensor.matmul(psGT[:], xs[64:98, isl], phat[64:98, CS[c]],
                                     start=True, stop=True, tile_position=(64, 0))
                    nc.scalar.sign(ATi[:], psGT[:])
                    nc.tensor.matmul(prodc[0:33, :], xtT[:, i, :], ATi[:],
                                     start=(i == 0), stop=(i == NT - 1),
                                     skip_group_check=True, tile_position=(0, 0))
                    nc.tensor.matmul(prodc[64:97, :], xtT[:, i, :], Ai[:],
                                     start=(i == 0), stop=(i == NT - 1),
                                     skip_group_check=True, tile_position=(0, 64))

                # ---- C(c): uf = (yA'+Sx)/(rs'+N), ub = yAT/cs -> xd rows ----
                nc.scalar.activation(vf[:, CS[c]], prodc[0:33, :], AF.Identity, bias=bSx[:])
                with nc.allow_low_precision(reason="4-byte recips"):
                    nc.vector.reciprocal(rr[:, CS[c]], vf[32:33, CS[c]])
                rrB = pf.tile([T, 512], F32, name="rrB", tag="ps")
                nc.tensor.matmul(rrB[:], ones1[:], rr[:, CS[c]], start=True, stop=True)
                nc.scalar.activation(vb[:, CS[c]], prodc[64:97, :], AF.Identity)
                with nc.allow_low_precision(reason="4-byte recips"):
                    nc.vector.reciprocal(cc[:, CS[c]], vb[32:33, CS[c]])
                nc.vector.tensor_tensor(xd[32:44, CS[c]], vf[0:T, CS[c]], rrB[:], ALU.mult)
                ccB = pf.tile([T, 512], F32, name="ccB", tag="ps")
                nc.tensor.matmul(ccB[:], ones1[:], cc[:, CS[c]], start=True, stop=True)
                nc.vector.tensor_tensor(xd[64:76, CS[c]], vb[0:T, CS[c]], ccB[:], ALU.mult)

                # ---- D(c): x_flat slices ----
                for k in range(6):
                    ps = pf.tile([128, 512], F32, name="psF", tag="ps")
                    nc.tensor.matmul(ps[:], wb[0:76, O_ES + k * 128:O_ES + (k + 1) * 128],
                                     xd[0:76, CS[c]], start=True, stop=True)
                    if k % 2 == 0:
                        nc.scalar.activation(zT[:, k, CS[c]], ps[:], AF.Relu)
                    else:
                        nc.vector.tensor_scalar(zT[:, k, CS[c]], ps[:], 0.0, None, ALU.max)

                # ---- E/F(c): encoder/decoder MLPs ----
                ps = pf.tile([HID2, 512], F32, name="psH1", tag="ps")
                for k in range(6):
                    nc.tensor.matmul(ps[:], wb[:, O_EW1 + k * 128:O_EW1 + (k + 1) * 128],
                                     zT[:, k, CS[c]], start=(k == 0), stop=(k == 5))
                if c % 2 == 0:
                    nc.scalar.activation(h1[:, CS[c]], ps[:], AF.Relu, bias=eb1)
                else:
                    nc.vector.tensor_scalar(h1[:, CS[c]], ps[:], eb1, 0.0, ALU.add, ALU.max)

                ps = pf.tile([HID2, 512], F32, name="psH2", tag="ps")
                nc.tensor.matmul(ps[:], wb[:, O_EW2:O_EW2 + HID2], h1[:, CS[c]],
                                 start=True, stop=True)
                if c % 2 == 1:
                    nc.scalar.activation(h2[:, CS[c]], ps[:], AF.Relu, bias=eb2)
                else:
                    nc.vector.tensor_scalar(h2[:, CS[c]], ps[:], eb2, 0.0, ALU.add, ALU.max)

                ps = pf.tile([H, 512], F32, name="psXe", tag="ps")
                nc.tensor.matmul(ps[:], wb[:, O_EW3:O_EW3 + H], h2[:, CS[c]],
                                 start=True, stop=False)
                for k in range(6):
                    nc.tensor.matmul(ps[:], wb[:, O_EPROJ + k * H:O_EPROJ + (k + 1) * H],
                                     zT[:, k, CS[c]], start=False, stop=(k == 5))
                if c % 2 == 0:
                    nc.scalar.activation(xe[:, CS[c]], ps[:], AF.Identity, bias=ebe)
                else:
                    nc.vector.tensor_scalar(xe[:, CS[c]], ps[:], ebe, None, ALU.add)

                ps = pf.tile([HID2, 512], F32, name="psG1", tag="ps")
                nc.tensor.matmul(ps[:], wb[0:H, O_DW1:O_DW1 + HID2], xe[:, CS[c]],
                                 start=True, stop=True)
                if c % 2 == 1:
                    nc.scalar.activation(g1[:, CS[c]], ps[:], AF.Relu, bias=db1)
                else:
                    nc.vector.tensor_scalar(g1[:, CS[c]], ps[:], db1, 0.0, ALU.add, ALU.max)

                ps = pf.tile([HID2, 512], F32, name="psG2", tag="ps")
                nc.tensor.matmul(ps[:], wb[:, O_DW2:O_DW2 + HID2], g1[:, CS[c]],
                                 start=True, stop=True)
                if c % 2 == 0:
                    nc.scalar.activation(g2[:, CS[c]], ps[:], AF.Relu, bias=db2)
                else:
                    nc.vector.tensor_scalar(g2[:, CS[c]], ps[:], db2, 0.0, ALU.add, ALU.max)

                ps = pf.tile([TOUT, 512], F32, name="psOd", tag="ps")
                nc.tensor.matmul(ps[:], wb[:, O_DW3:O_DW3 + TOUT], g2[:, CS[c]],
                                 start=True, stop=False)
                nc.tensor.matmul(ps[:], wb[0:H, O_DPROJ:O_DPROJ + TOUT], xe[:, CS[c]],
                                 start=False, stop=True)
                if c % 2 == 1:
                    nc.scalar.activation(od[:, CS[c]], ps[:], AF.Identity, bias=dbd)
                else:
                    nc.vector.tensor_scalar(od[:, CS[c]], ps[:], dbd, None, ALU.add)
                eng = (nc.gpsimd, nc.scalar, nc.sync)[c]
                eng.dma_start(out=d["out"].ap()[:, CS[c]], in_=od[:, CS[c]])
